# revision 56
# baseline (speedup 1.0000x reference)
"""Self-contained Trainium2 kernel for nn_AssemblyArrayComponent_9019431322130.

Data-parallel over batch: 16 samples -> 8 cores x 2 samples.
Host folds (w_in @ conv1 @ bn1) and (conv2 @ bn2) into plain matmuls
(stride==kernel convs are reshapes); device runs the whole net per core:
  GEMM1+gelu -> GEMM2+gelu -> linear attention -> FF -> Mamba-2 SSD (chunked,
  Q=128) -> gated RMS -> out proj -> RMS -> LN.
Activations live as [d, t] (feature on partition, t = 2*512 tokens sample-major).

Engine split (v2): PE also does mean-centering (C = I - 11^T/128) and the
depthwise conv (diag-weight taps into PSUM); Act does all activations with
bias operands and the rstd = exp(-0.5*ln(var+eps)) chain grouped into 5
act-table phases; Pool does the S2 row broadcasts + decay-diff adds; DVE
keeps bf16/SBUF ops in 2x/4x modes.
"""
import sys
sys.path.insert(0, '/opt/trn_rl_repo')
import numpy as np
import ml_dtypes

import concourse.bass as bass
import concourse.tile as tile
import concourse.mybir as mybir
from concourse import bacc, library_config
from concourse.bass_utils import run_bass_kernel_spmd

f32 = mybir.dt.float32
bf16 = mybir.dt.bfloat16
AF = mybir.ActivationFunctionType
OP = mybir.AluOpType
BF = ml_dtypes.bfloat16

# Guide the act-table chooser: the greedy pass picks the first set containing
# a function; hide exp/ln from their single-function sets so ln+exp phases
# resolve to natural_log_exp_and_others (set indices stay valid — only the
# candidate filter changes, and each emitted load is still fully charged).
import concourse.hw_specs as _hw_specs
from concourse import bacc as _bacc_mod
_ORIG_TABLES = _hw_specs.get_activation_tables


def _patched_tables(arch):
    out = {}
    for name, s in _ORIG_TABLES(arch).items():
        s2 = set(s)
        if name == 'exp_and_others':
            s2.discard(AF.Exp)
        if name == 'natural_log':
            s2.discard(AF.Ln)
        out[name] = s2
    return out


_bacc_mod.get_activation_tables = _patched_tables

B, L, E = 16, 16384, 16
H = 128
NH, DH = 4, 32
FF = 256
D_STATE, HEADDIM = 32, 32
D_INNER = 2 * H
NHEADS = 8
CONV_DIM = 320
DCONV = 4
LC = 512
BN_EPS = 1e-5
Q = 128          # SSD chunk
NCH = 4          # chunks per sample
BLOC = 2         # samples per core
T = BLOC * LC    # 1024 tokens per core


# (name, rows, cols_or_tuple, dtype-class)
WSPEC = [
    ("wW1", 128, 128, "b"), ("wW2", 128, (4, 128), "b"),
    ("wq", 128, 128, "b"), ("wk", 128, 128, "b"), ("wv", 128, 128, "b"),
    ("wo", 128, 128, "b"), ("ff1w", 128, 256, "b"), ("ff2w", 128, (2, 128), "b"),
    ("ipw", 128, 584, "b"), ("outw", 128, (2, 128), "b"),
    ("sel8", 8, 256, "b"), ("sel4", 4, 128, "b"),
    ("sel4T", 128, 4, "b"), ("mask01", 128, 128, "b"),
    ("onesm128", 128, 128, "b"), ("onesm256", 128, 128, "b"),
    ("cmat", 128, 128, "b"), ("cmato", 128, 128, "b"),
    ("convdiag", 128, (12, 128), "b"),
    ("eye", 128, 128, "b"), ("onecol", 128, 1, "b"),
    ("bqr", 1, 128, "b"), ("bkr", 1, 128, "b"), ("bvr", 1, 128, "b"),
    ("onesrowb", 1, 512, "b"),
    ("b1", 128, 1, "f"), ("b2", 128, 1, "f"),
    ("ln1g", 128, 1, "f"), ("ln1b", 128, 1, "f"),
    ("ln2g", 128, 1, "f"), ("ln2b", 128, 1, "f"),
    ("olng", 128, 1, "f"), ("olnb", 128, 1, "f"),
    ("rmsw", 128, 1, "f"), ("mnormw", 128, 2, "f"), ("bo", 128, 1, "f"),
    ("ff1b", 128, 2, "f"), ("ff2b", 128, 1, "f"),
    ("convb", 128, 3, "f"),
    ("dtbias", 8, 1, "f"), ("A2", 8, 1, "f"), ("Dexp", 128, 2, "f"),
    ("eyef", 128, 128, "f"),
    ("epsln", 128, 1, "f"), ("epsrms", 128, 1, "f"),
]
W_OFF = {}
WF_COLS = 0
WB_COLS = 0
for _nm, _r, _c, _d in WSPEC:
    _n = int(np.prod(_c)) if isinstance(_c, tuple) else _c
    if _d == "f":
        W_OFF[_nm] = WF_COLS; WF_COLS += _n
    else:
        W_OFF[_nm] = WB_COLS; WB_COLS += _n


def _ap(t_ap, offset_elems, dims):
    return bass.AP(t_ap.tensor, t_ap.offset + offset_elems, dims)


def build_nc():
    nc = bacc.Bacc('TRN2', target_bir_lowering=False, debug=False, num_devices=8)
    dram = {}

    def din(name, shape, dt):
        dram[name] = nc.dram_tensor(name, shape, dt, kind="ExternalInput")
        return dram[name]

    xT = din("xT", [128, 4096], bf16)
    wpackf = din("wpackf", [128, WF_COLS], f32)
    wpackb = din("wpackb", [128, WB_COLS], bf16)
    out_d = nc.dram_tensor("out", [128, 1024], bf16, kind="ExternalOutput")

    with tile.TileContext(nc) as tc:
        with (
            tc.tile_pool(name="wp", bufs=1) as wp,      # weights/consts
            tc.tile_pool(name="ap", bufs=1) as apool,   # persistent activations
            tc.tile_pool(name="tp", bufs=2) as tp,      # transients
            tc.tile_pool(name="pw", bufs=4, space="PSUM") as pw,   # wide psum
            tc.tile_pool(name="pb", bufs=2, space="PSUM") as pb,   # block psum
            tc.tile_pool(name="py", bufs=2, space="PSUM") as py,   # Y accum
        ):
            wpf = wp.tile([128, WF_COLS], f32, tag="wpf")
            nc.sync.dma_start(wpf[:], wpackf[:])
            wpb = wp.tile([128, WB_COLS], bf16, tag="wpb")
            xTs = apool.tile([128, 4096], bf16, tag="bigB", name="xTs")
            _c3 = WB_COLS // 3
            nc.sync.dma_start(wpb[:, 0:_c3], wpackb[:, 0:_c3])
            nc.sync.dma_start(wpb[:, _c3:2 * _c3], wpackb[:, _c3:2 * _c3])
            nc.sync.dma_start(wpb[:, 2 * _c3:], wpackb[:, 2 * _c3:])
            for i in range(8):
                nc.sync.dma_start(xTs[:, bass.ts(i, 512)], xT[:, bass.ts(i, 512)])
            # per-engine warm-ups: absorb the weight-DMA waits once per engine
            wa0 = tp.tile([1, 4], f32, tag="warm", bufs=1)
            nc.vector.tensor_copy(wa0[:], wpf[0:1, 0:4])
            wb0 = tp.tile([1, 4], bf16, tag="warm", bufs=1)
            nc.vector.tensor_copy(wb0[:], wpb[0:1, 0:4])
            wa1 = tp.tile([1, 4], f32, tag="warm", bufs=1)
            nc.scalar.copy(wa1[:], wpf[0:1, 0:4])
            wb1 = tp.tile([1, 4], bf16, tag="warm", bufs=1)
            nc.scalar.copy(wb1[:], wpb[0:1, 0:4])
            wg = tp.tile([2, 4], f32, tag="warm", bufs=1)
            nc.gpsimd.partition_broadcast(wg[:], wpf[0:1, 0:4])
            W = {"xT": xTs}
            for nm, rows, cols, dt in WSPEC:
                off = W_OFF[nm]
                buf = wpf if dt == "f" else wpb
                ncols = int(np.prod(cols)) if isinstance(cols, tuple) else cols
                apv = buf[0:rows, off:off + ncols]
                if isinstance(cols, tuple):
                    apv = apv.rearrange("p (a b) -> p a b", a=cols[0])
                W[nm] = apv

            # ---------------- GEMM1 + gelu ----------------
            h1 = apool.tile([128, 4096], bf16, tag="bigA", name="h1")
            for i in range(8):
                ps = pw.tile([128, 512], f32, tag="psw")
                nc.tensor.matmul(ps[:], W["wW1"][:], W["xT"][:, bass.ts(i, 512)],
                                 start=True, stop=True)
                nc.scalar.activation(h1[:, bass.ts(i, 512)], ps[:],
                                     AF.Gelu_apprx_tanh, bias=W["b1"][:, 0:1])

            # ---------------- GEMM2 + gelu -> h [128,1024] ----------------
            h_bf = apool.tile([128, 1024], bf16, tag="h_bf")
            for s in range(BLOC):
                ps = pw.tile([128, 512], f32, tag="psw")
                for k in range(4):
                    rhs = _ap(h1[:], s * 2048 + k, [list(h1[:].ap[0]), [4, 512]])
                    nc.tensor.matmul(ps[:], W["wW2"][:, k, :], rhs,
                                     start=(k == 0), stop=(k == 3))
                nc.scalar.activation(h_bf[:, bass.ts(s, 512)], ps[:],
                                     AF.Gelu_apprx_tanh, bias=W["b2"][:, 0:1])

            # ---------------- LayerNorm helper (centered via PE) ----------
            # affine=None returns the two normalized half tiles (g/b folded
            # into the consumer weights); affine=(g, b) applies them via Act.
            def layer_norm(x, g, b, eps, tagp="ln", cmat="cmat", affine=True,
                           out_dt=bf16):
                halves = []
                out = None
                if affine:
                    out = apool.tile([128, 1024], out_dt, tag=tagp + "_out")
                for hf in range(2):
                    xc = pw.tile([128, 512], f32, tag="psw")
                    nc.tensor.matmul(xc[:], W[cmat][:], x[:, bass.ts(hf, 512)],
                                     start=True, stop=True)
                    xcs = tp.tile([128, 512], bf16, tag="ln_xcs", bufs=2)
                    nc.vector.tensor_copy(xcs[:], xc[:])
                    sq = tp.tile([128, 512], bf16, tag="ln_sq", bufs=2)
                    nc.vector.tensor_tensor(out=sq[:], in0=xcs[:], in1=xcs[:],
                                            op=OP.mult)
                    eq = pw.tile([128, 512], f32, tag="psw")
                    nc.tensor.matmul(eq[:], W["onesm128"][:], sq[:],
                                     start=True, stop=True)
                    lnv = tp.tile([128, 512], f32, tag="ln_lnv", bufs=2)
                    nc.scalar.activation(lnv[:], eq[:], AF.Ln, bias=eps[:, 0:1])
                    rstd = tp.tile([128, 512], bf16, tag="ln_rstd", bufs=3)
                    nc.scalar.activation(rstd[:], lnv[:], AF.Exp, scale=-0.5)
                    t2 = tp.tile([128, 512], bf16, tag=tagp + "_t2", bufs=2)
                    nc.vector.tensor_tensor(out=t2[:], in0=xcs[:], in1=rstd[:],
                                            op=OP.mult)
                    if affine:
                        nc.vector.tensor_scalar(out=out[:, bass.ts(hf, 512)],
                                                in0=t2[:], scalar1=g[:, 0:1],
                                                scalar2=b[:, 0:1],
                                                op0=OP.mult, op1=OP.add)
                    halves.append(t2)
                return out, halves

            # ---------------- attention ----------------
            a_bf, _ = layer_norm(h_bf, W["ln1g"], W["ln1b"], W["epsln"],
                                 tagp="ln1")

            # q in [dq, t]
            q_bf = apool.tile([128, 1024], bf16, tag="q_bf")
            for hf in range(2):
                ps = pw.tile([128, 512], f32, tag="psw")
                nc.tensor.matmul(ps[:], W["wq"][:], a_bf[:, bass.ts(hf, 512)],
                                 start=True, stop=True)
                xm = tp.tile([128, 512], bf16, tag="xm")
                nc.vector.tensor_scalar(out=xm[:], in0=ps[:], scalar1=0.0,
                                        scalar2=None, op0=OP.min)
                em = tp.tile([128, 512], bf16, tag="em")
                nc.scalar.activation(em[:], xm[:], AF.Exp)
                nc.vector.scalar_tensor_tensor(
                    out=q_bf[:, bass.ts(hf, 512)], in0=ps[:], scalar=0.0,
                    in1=em[:], op0=OP.max, op1=OP.add)

            # k', v' in [t, d] tiles
            kT = apool.tile([128, 8, 128], bf16, tag="kT")
            vT = apool.tile([128, 8, 128], bf16, tag="vT")
            for half in range(2):
                psk = pw.tile([128, 512], f32, tag="psw")
                psv = pw.tile([128, 512], f32, tag="psw")
                for q4 in range(4):
                    tt = 4 * half + q4
                    nc.tensor.matmul(psk[:, bass.ts(q4, 128)],
                                     a_bf[:, bass.ts(tt, 128)], W["wk"][:],
                                     start=True, stop=True)
                    nc.tensor.matmul(psv[:, bass.ts(q4, 128)],
                                     a_bf[:, bass.ts(tt, 128)], W["wv"][:],
                                     start=True, stop=True)
                xm = tp.tile([128, 512], bf16, tag="xm")
                nc.vector.tensor_scalar(out=xm[:], in0=psk[:], scalar1=0.0,
                                        scalar2=None, op0=OP.min)
                em = tp.tile([128, 512], bf16, tag="em")
                nc.scalar.activation(em[:], xm[:], AF.Exp)
                nc.vector.scalar_tensor_tensor(
                    out=kT[:].rearrange("p a b -> p (a b)")[:, bass.ts(half, 512)],
                    in0=psk[:], scalar=0.0, in1=em[:], op0=OP.max, op1=OP.add)
                nc.scalar.copy(
                    vT[:].rearrange("p a b -> p (a b)")[:, bass.ts(half, 512)],
                    psv[:])

            # kv[d,e] per (b,h) stacked on partitions; ksum via ones rhs
            kv_sb, ksumM = [], []
            for s in range(BLOC):
                kvp = pb.tile([128, 32], f32, tag="psb")
                for hh in range(4):
                    for tt in range(4):
                        nc.tensor.matmul(
                            kvp[32 * hh:32 * hh + 32, :],
                            kT[:, 4 * s + tt, 32 * hh:32 * hh + 32],
                            vT[:, 4 * s + tt, 32 * hh:32 * hh + 32],
                            start=(tt == 0), stop=(tt == 3),
                            tile_position=(0, 32 * hh))
                kv = apool.tile([128, 32], bf16, tag=f"kv{s}")
                nc.scalar.copy(kv[:], kvp[:])
                kv_sb.append(kv)
                ksp = pb.tile([128, 1], f32, tag="psb")
                for tt in range(4):
                    nc.tensor.matmul(ksp[:], kT[:, 4 * s + tt, :], W["onecol"][:],
                                     start=(tt == 0), stop=(tt == 3))
                km = apool.tile([128, 4], bf16, tag=f"ksumM{s}")
                ksp_bc = _ap(ksp[:], 0, [list(ksp[:].ap[0]), [0, 4]])
                nc.vector.tensor_tensor(out=km[:], in0=ksp_bc,
                                        in1=W["sel4T"][:], op=OP.mult)
                ksumM.append(km)

            attnf = apool.tile([128, 1024], bf16, tag="attnf")
            for s in range(BLOC):
                den = pb.tile([4, 512], f32, tag="psb")
                nc.tensor.matmul(den[:], ksumM[s][:], q_bf[:, bass.ts(s, 512)],
                                 start=True, stop=True)
                zrb = tp.tile([4, 512], bf16, tag="zrb")
                with nc.allow_low_precision(reason="z feeds bf16 matmul rhs"):
                    nc.vector.reciprocal(zrb[:], den[:])
                zrx = pb.tile([128, 512], f32, tag="psb")
                nc.tensor.matmul(zrx[:], W["sel4"][:], zrb[:], start=True, stop=True)
                zrxs = tp.tile([128, 512], bf16, tag="zrxs")
                nc.scalar.copy(zrxs[:], zrx[:])
                atp = pw.tile([128, 512], f32, tag="psw")
                for hh in range(4):
                    nc.tensor.matmul(atp[32 * hh:32 * hh + 32, :],
                                     kv_sb[s][32 * hh:32 * hh + 32, :],
                                     q_bf[32 * hh:32 * hh + 32, bass.ts(s, 512)],
                                     start=True, stop=True,
                                     tile_position=(32 * hh, 32 * hh))
                nc.vector.tensor_tensor(out=attnf[:, bass.ts(s, 512)], in0=atp[:],
                                        in1=zrxs[:], op=OP.mult)

            h2_bf = apool.tile([128, 1024], bf16, tag="h2_bf")
            for hf in range(2):
                ps = pw.tile([128, 512], f32, tag="psw")
                nc.tensor.matmul(ps[:], W["wo"][:], attnf[:, bass.ts(hf, 512)],
                                 start=True, stop=True)
                nc.vector.scalar_tensor_tensor(
                    out=h2_bf[:, bass.ts(hf, 512)], in0=ps[:],
                    scalar=W["bo"][:, 0:1], in1=h_bf[:, bass.ts(hf, 512)],
                    op0=OP.add, op1=OP.add)

            # ---------------- FF ----------------
            f_bf, _ = layer_norm(h2_bf, W["ln2g"], W["ln2b"], W["epsln"],
                                 tagp="ln2")
            gff = apool.tile([128, 2, 1024], bf16, tag="bigA", name="gff")
            for mt in range(2):
                for hf in range(2):
                    ps = pw.tile([128, 512], f32, tag="psw")
                    nc.tensor.matmul(ps[:], W["ff1w"][:, bass.ts(mt, 128)],
                                     f_bf[:, bass.ts(hf, 512)],
                                     start=True, stop=True)
                    nc.scalar.activation(gff[:, mt, bass.ts(hf, 512)], ps[:],
                                         AF.Gelu_apprx_tanh,
                                         bias=W["ff1b"][:, mt:mt + 1])
            h3_bf = apool.tile([128, 1024], bf16, tag="h3_bf")
            for hf in range(2):
                ps = pw.tile([128, 512], f32, tag="psw")
                for kt in range(2):
                    nc.tensor.matmul(ps[:], W["ff2w"][:, kt, :],
                                     gff[:, kt, bass.ts(hf, 512)],
                                     start=(kt == 0), stop=(kt == 1))
                nc.vector.scalar_tensor_tensor(
                    out=h3_bf[:, bass.ts(hf, 512)], in0=ps[:],
                    scalar=W["ff2b"][:, 0:1], in1=h2_bf[:, bass.ts(hf, 512)],
                    op0=OP.add, op1=OP.add)

            # ---------------- Mamba: in_proj ----------------
            # m-tiles: 0,1 -> zg; 2,3 -> x channels; 4 -> B,C,dt
            zgs = apool.tile([128, 2, 1024], bf16, tag="bigB", name="zgs")
            xpad = apool.tile([128, 6, 515], bf16, tag="bigC", name="xpad")
            dtraw = apool.tile([8, 1024], f32, tag="dtraw")
            for hf in range(2):
                for mt in range(2):
                    ps = pw.tile([128, 512], f32, tag="psw")
                    nc.tensor.matmul(ps[:], W["ipw"][:, bass.ts(mt, 128)],
                                     h3_bf[:, bass.ts(hf, 512)],
                                     start=True, stop=True)
                    nc.scalar.activation(zgs[:, mt, bass.ts(hf, 512)], ps[:],
                                         AF.Silu)
                for ct in range(2):
                    ps = pw.tile([128, 512], f32, tag="psw")
                    nc.tensor.matmul(ps[:], W["ipw"][:, bass.ts(2 + ct, 128)],
                                     h3_bf[:, bass.ts(hf, 512)],
                                     start=True, stop=True)
                    nc.gpsimd.memset(xpad[:, 3 * hf + ct, 0:3], 0.0)
                    nc.vector.tensor_copy(xpad[:, 3 * hf + ct, 3:515], ps[:])
                ps = pw.tile([72, 512], f32, tag="psw")
                nc.tensor.matmul(ps[:], W["ipw"][:, 512:584],
                                 h3_bf[:, bass.ts(hf, 512)], start=True, stop=True)
                nc.gpsimd.memset(xpad[0:64, 3 * hf + 2, 0:3], 0.0)
                nc.vector.tensor_copy(xpad[0:64, 3 * hf + 2, 3:515], ps[0:64, :])
                nc.vector.tensor_copy(dtraw[:, bass.ts(hf, 512)], ps[64:72, :])

            # depthwise causal conv on PE (diag-weight taps) + silu(+bias)
            xbcs = apool.tile([128, 6, 512], bf16, tag="xbcs")
            for s in range(BLOC):
                for ct in range(3):
                    rows = 128 if ct < 2 else 64
                    cps = pw.tile([128, 512], f32, tag="psw")
                    for k in range(4):
                        nc.tensor.matmul(
                            cps[0:rows, :],
                            W["convdiag"][0:rows, 4 * ct + k, 0:rows],
                            xpad[0:rows, 3 * s + ct, k:512 + k],
                            start=(k == 0), stop=(k == 3))
                    nc.scalar.activation(xbcs[0:rows, 3 * s + ct, :],
                                         cps[0:rows, :], AF.Silu,
                                         bias=W["convb"][0:rows, ct:ct + 1])

            # softplus(dtraw + dt_bias) over [8,1024]
            # = max(x,0) + ln(1 + exp(-|x|)); rotating spbuf tag (3 bufs)
            absx = tp.tile([8, 1024], f32, tag="spbuf", bufs=3)
            nc.scalar.activation(absx[:], dtraw[:], AF.Abs,
                                 bias=W["dtbias"][0:8, 0:1])
            espx = tp.tile([8, 1024], f32, tag="spbuf", bufs=3)
            nc.scalar.activation(espx[:], absx[:], AF.Exp, scale=-1.0)
            ep1 = tp.tile([8, 1024], f32, tag="spbuf", bufs=3)
            nc.vector.tensor_scalar(out=ep1[:], in0=espx[:], scalar1=1.0,
                                    scalar2=None, op0=OP.add)
            lnpart = tp.tile([8, 1024], f32, tag="spbuf", bufs=3)
            nc.scalar.activation(lnpart[:], ep1[:], AF.Ln)
            xplus = tp.tile([8, 1024], f32, tag="spbuf", bufs=3)
            nc.vector.tensor_scalar(out=xplus[:], in0=dtraw[:],
                                    scalar1=W["dtbias"][0:8, 0:1], scalar2=0.0,
                                    op0=OP.add, op1=OP.max)
            dt2a = tp.tile([8, 1024], f32, tag="spbuf", bufs=3)
            nc.vector.tensor_tensor(out=dt2a[:], in0=xplus[:], in1=lnpart[:],
                                    op=OP.add)

            # dt products (all [8, 1024]: heads on partitions, samples on free)
            dtA8 = tp.tile([8, 1024], f32, tag="spbuf", bufs=3)
            nc.vector.tensor_scalar(out=dtA8[:], in0=dt2a[:], scalar1=W["A2"][:, 0:1],
                                    scalar2=None, op0=OP.mult)
            dt2bf = apool.tile([8, 1024], bf16, tag="dt2bf")
            nc.vector.tensor_copy(dt2bf[:], dt2a[:])

            # chunk-local inclusive cumsum S2 [8, 1024]; global chunk g = s*4+c
            S2 = apool.tile([8, 1024], f32, tag="S2")
            for g in range(8):
                nc.vector.tensor_tensor_scan(
                    out=S2[:, bass.ts(g, 128)], data0=dtA8[:, bass.ts(g, 128)],
                    data1=dtA8[:, bass.ts(g, 128)], initial=0.0,
                    op0=OP.add, op1=OP.bypass)

            # transposes of S2 chunks -> S2T [128, 8*8] (cols g*8+h), negated
            S2T = apool.tile([128, 64], f32, tag="S2T")
            for g in range(8):
                pt = pb.tile([128, 8], f32, tag="psb")
                nc.tensor.transpose(pt[:], S2[:, bass.ts(g, 128)],
                                    W["eyef"][0:8, 0:8])
                nc.vector.tensor_scalar(out=S2T[:, bass.ts(g, 8)], in0=pt[:],
                                        scalar1=-1.0, scalar2=None, op0=OP.mult)

            # S2 rows bounced via DRAM (re-read any row at partition 0)
            S2d = nc.dram_tensor("S2d", [8, 1024], f32)
            nc.sync.dma_start(S2d[:], S2[:])

            # cp = exp(S2); wend = exp(S_end - S2)
            cp8 = apool.tile([8, 1024], bf16, tag="cp8")
            nc.scalar.activation(cp8[:], S2[:], AF.Exp)
            wl = tp.tile([8, 1024], f32, tag="wl", bufs=1)
            send_ap = _ap(S2[:], 127, [list(S2[:].ap[0]), [128, 8], [0, 128]])
            nc.vector.tensor_tensor(out=wl[:].rearrange("p (c j) -> p c j", c=8),
                                    in0=send_ap,
                                    in1=S2[:].rearrange("p (c j) -> p c j", c=8),
                                    op=OP.subtract)
            wend_bf = apool.tile([8, 1024], bf16, tag="wend_bf")
            nc.scalar.activation(wend_bf[:], wl[:], AF.Exp)
            # wendT [128, 8*8] (cols g*8+h)
            wendT = apool.tile([128, 64], bf16, tag="wendT")
            for g in range(8):
                pt = pb.tile([128, 8], bf16, tag="psb")
                nc.tensor.transpose(pt[:], wend_bf[:, bass.ts(g, 128)],
                                    W["eye"][0:8, 0:8])
                nc.scalar.copy(wendT[:, bass.ts(g, 8)], pt[:])

            # dt broadcast + xdt
            xdt = apool.tile([128, 6, 512], bf16, tag="bigD", name="xdt")
            for s in range(BLOC):
                for jt in range(2):
                    dx = pw.tile([128, 512], f32, tag="psw")
                    nc.tensor.matmul(dx[:], W["sel8"][:, bass.ts(jt, 128)],
                                     dt2bf[:, bass.ts(s, 512)], start=True, stop=True)
                    nc.vector.tensor_tensor(out=xdt[:, 3 * s + jt, :],
                                            in0=xbcs[:, 3 * s + jt, :], in1=dx[:],
                                            op=OP.mult)

            # xdtT [t_local, (tb, ch256)] per sample: batched transposes
            xdtT = [apool.tile([128, 4, 256], bf16, tag=f"xdtT{s}", name=f"xdtT{s}")
                    for s in range(BLOC)]
            for s in range(BLOC):
                for jt in range(2):
                    ptp = pb.tile([128, 512], bf16, tag="psb")
                    for tb in range(4):
                        nc.tensor.transpose(
                            ptp[:, bass.ts(tb, 128)],
                            xdt[:, 3 * s + jt, bass.ts(tb, 128)], W["eye"][:])
                    out_ap = _ap(xdtT[s][:], jt * 128,
                                 [list(xdtT[s][:].ap[0]), [256, 4], [1, 128]])
                    nc.scalar.copy(
                        out_ap, ptp[:].rearrange("p (a b) -> p a b", a=4))

            # xdtw = xdtT * wend (per-head, free-broadcast over p)
            xdtw = [apool.tile([128, 4, 256], bf16, tag=f"xdtw{s}", name=f"xdtw{s}")
                    for s in range(BLOC)]
            for s in range(BLOC):
                for tb in range(4):
                    wap = _ap(wendT[:], (4 * s + tb) * 8,
                              [list(wendT[:].ap[0]), [1, 8], [0, 32]])
                    nc.gpsimd.tensor_tensor(
                        out=xdtw[s][:, tb, :].rearrange("p (h q) -> p h q", h=8),
                        in0=xdtT[s][:, tb, :].rearrange("p (h q) -> p h q", h=8),
                        in1=wap, op=OP.mult)

            # Bm/Cm at partition base 0 [32, 512] per sample
            Bm_sb = [apool.tile([32, 512], bf16, tag=f"Bm{s}", name=f"Bm{s}")
                     for s in range(BLOC)]
            Cm_sb = [apool.tile([32, 512], bf16, tag=f"Cm{s}", name=f"Cm{s}")
                     for s in range(BLOC)]
            for s in range(BLOC):
                nc.scalar.copy(Bm_sb[s][:], xbcs[0:32, 3 * s + 2, :])
                nc.scalar.copy(Cm_sb[s][:], xbcs[32:64, 3 * s + 2, :])

            # BT [t_local, (tb, n32)] per sample
            BT = [apool.tile([128, 4, 32], bf16, tag=f"BT{s}", name=f"BT{s}")
                  for s in range(BLOC)]
            for s in range(BLOC):
                for tb in range(4):
                    pt = pb.tile([128, 128], bf16, tag="psb")
                    nc.tensor.transpose(pt[:], xbcs[:, 3 * s + 2, bass.ts(tb, 128)],
                                        W["eye"][:])
                    nc.scalar.copy(BT[s][:, tb, :], pt[:, 0:32])

            # G premasked per (s, c)
            GTm = [apool.tile([128, 4, 128], bf16, tag=f"GTm{s}", name=f"GTm{s}")
                   for s in range(BLOC)]
            for s in range(BLOC):
                for c in range(NCH):
                    gp = pb.tile([128, 128], f32, tag="psb")
                    nc.tensor.matmul(gp[:], Bm_sb[s][:, bass.ts(c, 128)],
                                     Cm_sb[s][:, bass.ts(c, 128)],
                                     start=True, stop=True)
                    nc.vector.tensor_tensor(out=GTm[s][:, c, :], in0=gp[:],
                                            in1=W["mask01"][:], op=OP.mult)

            # decay matrices per head h (both samples at once):
            #   Sbc[p, (g,t)] = S2[h, (g,t)] via partition-broadcast DMA read;
            #   dr = Sbc - S2T_col; LT = exp (Act); LTm = min(LT,1) (TS 4x);
            #   MT[s][h] = LTm * GTm (TT 2x).
            MT = [[apool.tile([128, 4, 128], bf16, tag=f"MT_{hh}",
                              name=f"MT{s}_{hh}")
                   for hh in range(NHEADS)] for s in range(BLOC)]
            for s in range(BLOC):
                for hh in range(NHEADS):
                    Sbc = tp.tile([128, 512], f32, tag="Sbc", bufs=2)
                    base = S2d[hh, bass.ts(s, 512)]
                    bcast = bass.AP(base.tensor, base.offset,
                                    [[0, 128], [1, 512]])
                    nc.sync.dma_start(Sbc[:], bcast)
                    scol = _ap(S2T[:], 32 * s + hh,
                               [list(S2T[:].ap[0]), [8, 4], [0, 128]])
                    eng = nc.vector if hh % 2 == 0 else nc.gpsimd
                    meng = nc.gpsimd if hh in (1, 3, 5) else nc.vector
                    dr = tp.tile([128, 4, 128], f32, tag="dr", bufs=2)
                    eng.tensor_tensor(
                        out=dr[:],
                        in0=Sbc[:].rearrange("p (c j) -> p c j", c=4),
                        in1=scol, op=OP.add)
                    LT = tp.tile([128, 512], bf16, tag="LT", bufs=2)
                    nc.scalar.activation(LT[:],
                                         dr[:].rearrange("p c j -> p (c j)"),
                                         AF.Exp)
                    # exp(min(x,0)) == min(exp(x),1); GTm has the causal zeros
                    LTm = tp.tile([128, 512], bf16, tag="LTm", bufs=2)
                    nc.vector.tensor_scalar(
                        out=LTm[:], in0=LT[:],
                        scalar1=1.0, scalar2=None, op0=OP.min)
                    meng.tensor_tensor(
                        out=MT[s][hh][:],
                        in0=LTm[:].rearrange("p (c j) -> p c j", c=4),
                        in1=GTm[s][:], op=OP.mult)

            # SSD main loop per sample
            y2 = apool.tile([128, 6, 512], bf16, tag="bigD", name="y2")
            for s in range(BLOC):
                Yp = [py.tile([128, 512], f32, tag="Yp", name="Yp")
                      for _ in range(2)]
                state = apool.tile([32, 256], bf16, tag=f"st_{s}_0")
                nc.vector.memset(state[:], 0.0)
                for c in range(NCH):
                    for hh in range(NHEADS):
                        jt, hq = hh // 4, hh % 4
                        nc.tensor.matmul(
                            Yp[jt][32 * hq:32 * hq + 32, bass.ts(c, 128)],
                            xdtT[s][:, c, 32 * hh:32 * hh + 32],
                            MT[s][hh][:, c, :],
                            start=True, stop=False,
                            tile_position=(0, 32 * hq))
                        nc.tensor.matmul(
                            Yp[jt][32 * hq:32 * hq + 32, bass.ts(c, 128)],
                            state[:, 32 * hh:32 * hh + 32],
                            Cm_sb[s][:, bass.ts(c, 128)],
                            start=False, stop=True,
                            tile_position=(0, 32 * hq))
                    # chunk state: T_c then recurrence
                    if c < NCH - 1:
                        Tp = pb.tile([32, 256], f32, tag="psb")
                        nc.tensor.matmul(Tp[:], BT[s][:, c, :], xdtw[s][:, c, :],
                                         start=True, stop=True)
                        aend8 = tp.tile([8, 32], bf16, tag="aend8")
                        aap = _ap(cp8[:], s * 512 + c * 128 + 127,
                                  [list(cp8[:].ap[0]), [0, 32]])
                        nc.vector.tensor_copy(aend8[:], aap)
                        aendB = pb.tile([32, 256], f32, tag="psb")
                        nc.tensor.matmul(aendB[:], aend8[:], W["sel8"][:],
                                         start=True, stop=True,
                                         tile_position=(0, 0))
                        st_tmp = tp.tile([32, 256], bf16, tag="st_tmp")
                        nc.vector.tensor_tensor(out=st_tmp[:], in0=state[:],
                                                in1=aendB[:], op=OP.mult)
                        state2 = apool.tile([32, 256], bf16, tag=f"st_{s}_{c + 1}")
                        nc.vector.tensor_tensor(out=state2[:], in0=st_tmp[:],
                                                in1=Tp[:], op=OP.add)
                        state = state2
                # post-scale by cp and add D_skip * xs
                for jt in range(2):
                    cx = pw.tile([128, 512], f32, tag="psw")
                    nc.tensor.matmul(cx[:], W["sel8"][:, bass.ts(jt, 128)],
                                     cp8[:, bass.ts(s, 512)], start=True, stop=True)
                    cxs = tp.tile([128, 512], bf16, tag="cpx_sb")
                    nc.scalar.copy(cxs[:], cx[:])
                    yt = tp.tile([128, 512], bf16, tag="yt")
                    nc.vector.tensor_tensor(out=yt[:], in0=Yp[jt][:], in1=cxs[:],
                                            op=OP.mult)
                    nc.vector.scalar_tensor_tensor(
                        out=y2[:, 3 * s + jt, :], in0=xbcs[:, 3 * s + jt, :],
                        scalar=W["Dexp"][:, jt:jt + 1], in1=yt[:],
                        op0=OP.mult, op1=OP.add)

            # gated RMS (over 256) then out proj
            yn = apool.tile([128, 6, 512], bf16, tag="bigC", name="yn")
            for s in range(BLOC):
                yz = [tp.tile([128, 512], bf16, tag="yz", name="yz")
                      for _ in range(2)]
                sqz = [tp.tile([128, 512], bf16, tag="sqz", name="sqz")
                       for _ in range(2)]
                for jt in range(2):
                    nc.vector.tensor_tensor(out=yz[jt][:], in0=y2[:, 3 * s + jt, :],
                                            in1=zgs[:, jt, bass.ts(s, 512)],
                                            op=OP.mult)
                    nc.vector.tensor_tensor(out=sqz[jt][:], in0=yz[jt][:],
                                            in1=yz[jt][:], op=OP.mult)
                eq = pw.tile([128, 512], f32, tag="psw")
                for jt in range(2):
                    nc.tensor.matmul(eq[:], W["onesm256"][:], sqz[jt][:],
                                     start=(jt == 0), stop=(jt == 1))
                lnv = tp.tile([128, 512], f32, tag="mn_lnv", bufs=1)
                nc.scalar.activation(lnv[:], eq[:], AF.Ln,
                                     bias=W["epsrms"][:, 0:1])
                rstd = tp.tile([128, 512], bf16, tag="ln_rstd", bufs=3)
                nc.scalar.activation(rstd[:], lnv[:], AF.Exp, scale=-0.5)
                for jt in range(2):
                    nc.vector.scalar_tensor_tensor(
                        out=yn[:, 3 * s + jt, :], in0=yz[jt][:],
                        scalar=W["mnormw"][:, jt:jt + 1], in1=rstd[:],
                        op0=OP.mult, op1=OP.mult)

            # out-proj + residual + rms_w + oln centering fused on PE:
            # xc = cmato @ (outw @ yn + h3)  (outw2 = outw @ cmato host-side;
            # the final rms cancels inside the output LN — per-token scale is
            # LN-invariant)
            yfin = apool.tile([128, 1024], bf16, tag="yfin")
            for s in range(BLOC):
                xc = pw.tile([128, 512], f32, tag="psw")
                for kt in range(2):
                    nc.tensor.matmul(xc[:], W["outw"][:, kt, :],
                                     yn[:, 3 * s + kt, :],
                                     start=(kt == 0), stop=False)
                nc.tensor.matmul(xc[:], W["cmato"][:],
                                 h3_bf[:, bass.ts(s, 512)],
                                 start=False, stop=True)
                xcs = tp.tile([128, 512], bf16, tag="ln_xcs", bufs=2)
                nc.vector.tensor_copy(xcs[:], xc[:])
                sq = tp.tile([128, 512], bf16, tag="ln_sq", bufs=2)
                nc.vector.tensor_tensor(out=sq[:], in0=xcs[:], in1=xcs[:],
                                        op=OP.mult)
                eq = pw.tile([128, 512], f32, tag="psw")
                nc.tensor.matmul(eq[:], W["onesm128"][:], sq[:],
                                 start=True, stop=True)
                lnv = tp.tile([128, 512], f32, tag="ln_lnv", bufs=2)
                nc.scalar.activation(lnv[:], eq[:], AF.Ln,
                                     bias=W["epsln"][:, 0:1])
                rstd = tp.tile([128, 512], bf16, tag="ln_rstd", bufs=3)
                nc.scalar.activation(rstd[:], lnv[:], AF.Exp, scale=-0.5)
                t2 = tp.tile([128, 512], bf16, tag="oln_t2", bufs=2)
                nc.vector.tensor_tensor(out=t2[:], in0=xcs[:], in1=rstd[:],
                                        op=OP.mult)
                nc.vector.tensor_scalar(out=yfin[:, bass.ts(s, 512)],
                                        in0=t2[:], scalar1=W["olng"][:, 0:1],
                                        scalar2=W["olnb"][:, 0:1],
                                        op0=OP.mult, op1=OP.add)
                nc.sync.dma_start(out_d[:, bass.ts(s, 512)],
                                  yfin[:, bass.ts(s, 512)])

    nc.compile()
    return nc


# ---------------- host side ----------------
_CACHE = {}


def _prep(inputs):
    d = {k: np.asarray(v, np.float32) for k, v in inputs.items()}
    inv = 1.0 / np.sqrt(1.0 + BN_EPS)
    W1 = np.einsum('ei,oik->keo', d['w_in'], d['conv1_w']).reshape(128, H)
    b1v = np.einsum('i,oik->o', d['b_in'], d['conv1_w'])
    s1 = d['bn1_g'] * inv
    W1 = W1 * s1[None, :]
    b1v = b1v * s1 + d['bn1_b']
    W2 = np.transpose(d['conv2_w'], (2, 1, 0)) * (d['bn2_g'] * inv)[None, None, :]
    W2sb = np.ascontiguousarray(np.transpose(W2, (1, 0, 2)))          # [i,k,o]
    ff2sb = np.ascontiguousarray(d['ff2_w'].reshape(2, 128, 128).transpose(1, 0, 2))
    _cm = (np.eye(128) - np.full((128, 128), 1.0 / 128)) @ np.diag(d['rms_w'])
    _ow2 = d['out_w'] @ _cm
    outsb = np.ascontiguousarray(_ow2.reshape(2, 128, 128).transpose(1, 0, 2))
    cb = np.zeros((128, 3), np.float32)
    cdiag = np.zeros((128, 12, 128), np.float32)
    for ct in range(3):
        rows = 128 if ct < 2 else 64
        cb[:rows, ct] = d['conv_b'][ct * 128:ct * 128 + rows]
        for k in range(4):
            w = d['conv_w'][ct * 128:ct * 128 + rows, k]
            cdiag[:rows, ct * 4 + k, :rows] = np.diag(w)
    A = -np.exp(d['A_log'])
    sel8 = np.zeros((8, 256), np.float32)
    for m in range(256):
        sel8[m // 32, m] = 1.0
    sel4 = np.zeros((4, 128), np.float32)
    for m in range(128):
        sel4[m // 32, m] = 1.0
    mask01 = (np.arange(128)[:, None] <= np.arange(128)[None, :]).astype(np.float32)
    Dexp = np.zeros((128, 2), np.float32)
    mw = np.zeros((128, 2), np.float32)
    for jt in range(2):
        for r in range(128):
            Dexp[r, jt] = d['D_skip'][4 * jt + r // 32]
            mw[r, jt] = d['mnorm_w'][jt * 128 + r]
    cmat = (np.eye(128) - np.full((128, 128), 1.0 / 128)).astype(np.float32)
    cmato = cmat @ np.diag(d['rms_w'])
    col = lambda v: np.ascontiguousarray(v.reshape(-1, 1), dtype=np.float32)
    vals = {
        'wW1': W1.astype(BF), 'b1': col(b1v),
        'wW2': W2sb.astype(BF), 'b2': col(d['bn2_b']),
        'ln1g': col(d['ln1_g']), 'ln1b': col(d['ln1_b']),
        'ln2g': col(d['ln2_g']), 'ln2b': col(d['ln2_b']),
        'olng': col(d['oln_g']), 'olnb': col(d['oln_b']),
        'rmsw': col(d['rms_w']), 'mnormw': mw,
        'wq': d['wq'].astype(BF),
        'wk': d['wk'].astype(BF),
        'wv': d['wv'].astype(BF),
        'bqr': np.zeros((1, 128), BF),
        'bkr': np.zeros((1, 128), BF),
        'bvr': np.zeros((1, 128), BF),
        'onesrowb': np.ones((1, 512), BF),
        'wo': d['wo'].astype(BF), 'bo': col(d['bo']),
        'ff1w': d['ff1_w'].astype(BF),
        'ff1b': np.ascontiguousarray(d['ff1_b'].reshape(2, 128).T),
        'ff2w': ff2sb.astype(BF), 'ff2b': col(d['ff2_b']),
        'ipw': d['in_proj_w'].astype(BF),
        'convb': cb, 'convdiag': cdiag.astype(BF),
        'dtbias': col(d['dt_bias']), 'A2': col(A),
        'Dexp': Dexp, 'outw': outsb.astype(BF),
        'sel8': sel8.astype(BF), 'sel4': sel4.astype(BF),
        'sel4T': np.ascontiguousarray(sel4.T).astype(BF),
        'mask01': mask01.astype(BF),
        'onesm128': np.full((128, 128), 1.0 / 128, BF),
        'onesm256': np.full((128, 128), 1.0 / 256, BF),
        'cmat': cmat.astype(BF), 'cmato': cmato.astype(BF),
        'eye': np.eye(128, dtype=BF),
        'eyef': np.eye(128, dtype=np.float32),
        'onecol': np.ones((128, 1), BF),
        'epsln': np.full((128, 1), 1e-5, np.float32),
        'epsrms': np.full((128, 1), 1e-6, np.float32),
    }
    wpackf = np.zeros((128, WF_COLS), np.float32)
    wpackb = np.zeros((128, WB_COLS), BF)
    for nm, rows, cols, dt in WSPEC:
        ncols = int(np.prod(cols)) if isinstance(cols, tuple) else cols
        v = np.asarray(vals[nm]).reshape(rows, ncols)
        off = W_OFF[nm]
        if dt == "f":
            wpackf[0:rows, off:off + ncols] = v
        else:
            wpackb[0:rows, off:off + ncols] = v
    wmap = {'wpackf': wpackf, 'wpackb': wpackb}
    return wmap


def kernel(**inputs):
    if 'nc' not in _CACHE:
        _CACHE['nc'] = build_nc()
    nc = _CACHE['nc']
    wmap = _prep(inputs)
    x = np.asarray(inputs['x'], np.float32)
    in_maps = []
    for core in range(8):
        xs = x[2 * core:2 * core + 2].reshape(2, 2048, 128)
        xTv = np.ascontiguousarray(xs.transpose(2, 0, 1).reshape(128, 4096))
        m = dict(wmap)
        m['xT'] = xTv.astype(BF)
        in_maps.append(m)
    res = run_bass_kernel_spmd(nc, in_maps, core_ids=list(range(8)))
    outs = []
    for core in range(8):
        o = np.asarray(res.results[core]['out'], np.float32)   # [128, 1024]
        outs.append(np.ascontiguousarray(o.T.reshape(2, 512, 128)))
    return np.concatenate(outs, 0).astype(np.float32)


if __name__ == '__main__':
    rng = np.random.default_rng(0)
    x = rng.standard_normal((B, L, E)).astype(np.float32)
    print("built module ok")
